# revision 29
# baseline (speedup 1.0000x reference)
"""DirGATConv on 8 Trainium2 NeuronCores (Bass/Tile).

Problem: nn_DirGATConv  (N=50000 nodes, E=800000 edges, DIN=128, DOUT=64)
    out = 0.5 * GATConv(x, src->dst, W1) + 0.5 * GATConv(x, dst->src, W2)

v2 design (zero collectives):
  * Nodes are grouped into 128-row chunks assigned to cores in contiguous
    ranges.  Conv1 groups edges by dst, conv2 by src; each core owns all
    edges whose group node is in its range and emits its output rows.
  * Phase A (replicated): one fused matmul per chunk produces per-conv
    gather tables Uc[n] = [xwc(64) | 1 | a_s_c | junk] (256B rows, f16).
    PSUM->SBUF copies run on the Activation engine.  Each core additionally
    writes its own chunks' attention-dst values into a core-local table A2
    laid out per super-chunk (SCN chunks + 128 zero rows), rows
    [a_d1 | a_d2 | junk] (256B); its xT slice arrives as a per-core input.
  * Phase B per edge slot: one U gather (by the non-group endpoint) and one
    A2 gather (by the group endpoint, super-chunk-relative index) deliver
    everything per edge: the score u = a_s + a_d adds two gathered columns,
    w = Exp(Lrelu(u)) runs on the Activation engine, and the scaled one-hot
    for the PE scatter is a single fused tensor_scalar
    (iota == dl) * w running at the DVE 4x rate.  The matmul rhs is the raw
    gathered row [xw | 1], so PSUM accumulates [w*xw | w] per chunk and the
    softmax normalization happens once at finalize.  Pad slots point at A2
    zero rows -> zero one-hot -> no contribution.
  * The SWDGE ring is enlarged 4x (64KB/partition) so gathers run in up to
    4096-descriptor calls, amortizing per-call Pool-engine overhead.
"""

import math

import numpy as np

import concourse.bass as bass
import concourse.mybir as mybir
import concourse.tile as tile
from concourse import bacc, bass_utils
from concourse._compat import with_exitstack

# ---------------------------------------------------------------- constants
N = 50000
E = 800000
DIN = 128
DOUT = 64
ALPHA = 0.5
NEG_SLOPE = 0.2
NCORES = 8
P = 128

G_REAL = math.ceil(N / P)              # 391 real node chunks
CPC = math.ceil(G_REAL / NCORES)       # 49 chunk slots per core
G = NCORES * CPC                       # 392 padded chunk slots
NT = G * P                             # 50176 padded node count
NPC = CPC * P                          # 6272 nodes per core
BANK = 32768                           # int16 gather-index bank size
RND = 2                                # node tiles per PSUM round in phase A
SCN = 3                                # chunks per gather super-chunk
NSC = math.ceil(CPC / SCN)             # super-chunks per core
AROWS = SCN * P + P                    # A2 rows per super-chunk (+zero pad)
DMA_SCRATCH = 16384                    # SWDGE ring: 1024 descriptors
GMAXB = (DMA_SCRATCH // 16) // P       # max 128-blocks per gather call
SASCOL = 65                            # a_s column in U rows

f32 = mybir.dt.float32
f16 = mybir.dt.float16
i16 = mybir.dt.int16

VARIANT = "full"    # "full" | "gathers" | "phasea"  (perf-bisect variants)

_CACHE = {}


def _wrap16(a):
    """[n*16] -> dma_gather idx layout [128, n] (16-wrap, replicated x8)."""
    n = a.shape[-1] // 16
    w = a.reshape(n, 16).T
    return np.ascontiguousarray(np.tile(w, (8, 1)))


# ------------------------------------------------------------ host preprocess
def _edge_arrays(key, gidx):
    """Bin edges by group node ("key"), 128 nodes per chunk; within a chunk
    split edges by gather-node bank (<BANK or >=BANK) into two slot spaces.
    Slot i of a span maps to partition i%128, block i//128.  Returns per-bank
    (U-idx wrapped, dl) + block counts, and the per-conv merged A2-relative
    index array (per super-chunk: bank-a blocks then bank-b blocks)."""
    order = np.lexsort((gidx, gidx >= BANK, key // P))
    key_s = key[order].astype(np.int64)
    gid_s = gidx[order].astype(np.int64)
    bank_s = (gid_s >= BANK).astype(np.int64)

    chunk = key_s // P
    cb = chunk * 2 + bank_s
    counts = np.bincount(cb, minlength=2 * G).reshape(G, 2)
    kb = -(-counts // P)                                  # [G, 2]
    kbA = np.maximum(kb[:, 0].reshape(NCORES, CPC).max(axis=0), 1)
    kbB = kb[:, 1].reshape(NCORES, CPC).max(axis=0)
    start = np.zeros(2 * G + 1, np.int64)
    start[1:] = np.cumsum(counts.reshape(-1))
    within = np.arange(key_s.size, dtype=np.int64) - start[cb]
    core = chunk // CPC
    slot = chunk % CPC
    per_bank = []
    for b, kbu in ((0, kbA), (1, kbB)):
        BO = np.zeros(CPC + 1, np.int64)
        BO[1:] = np.cumsum(kbu)
        ktot = int(BO[-1])
        sel = bank_s == b
        s = BO[slot[sel]] * P + within[sel]
        idx = np.zeros((NCORES, max(ktot, 1) * P), np.int16)
        dl = np.full((NCORES, max(ktot, 1) * P), -1.0, np.float32)
        idx[core[sel], s] = (gid_s[sel] - b * BANK).astype(np.int16)
        dl[core[sel], s] = (key_s[sel] - chunk[sel] * P).astype(np.float32)
        idxw = np.stack([_wrap16(idx[k]) for k in range(NCORES)])
        dl2 = np.ascontiguousarray(
            dl.reshape(NCORES, max(ktot, 1), P).transpose(0, 2, 1))
        per_bank.append((idxw, dl2, tuple(int(x) for x in kbu)))
    return per_bank


def _preprocess(x, edge_index, W1, att_src1, att_dst1, b1,
                W2, att_src2, att_dst2, b2):
    src = np.asarray(edge_index[0], np.int64)
    dst = np.asarray(edge_index[1], np.int64)
    loops = np.arange(N, dtype=np.int64)
    all_src = np.concatenate([src, loops])
    all_dst = np.concatenate([dst, loops])

    c1 = _edge_arrays(all_dst, all_src)   # conv1: group dst, gather src
    c2 = _edge_arrays(all_src, all_dst)   # conv2: group src, gather dst

    xT = np.zeros((DIN, NT), np.float16)
    xT[:, :N] = np.asarray(x, np.float32).T.astype(np.float16)

    wfull = np.zeros((DIN, 192), np.float32)
    wfull[:, 0:64] = W1
    wfull[:, 64:128] = W2
    wfull[:, 128] = W1 @ att_src1
    wfull[:, 129] = W1 @ att_dst1
    wfull[:, 130] = W2 @ att_src2
    wfull[:, 131] = W2 @ att_dst2
    wfull = wfull.astype(np.float16)
    adw = np.stack([W1 @ att_dst1, W2 @ att_dst2], axis=1).astype(np.float16)

    iota = np.broadcast_to(np.arange(P, dtype=np.float16), (P, P)).copy()
    bcomb = np.broadcast_to(
        ((1.0 - ALPHA) * b1 + ALPHA * b2).astype(np.float32), (P, DOUT)).copy()

    common = dict(xT=xT, wfull=wfull, adw=adw, iota=iota, bcomb=bcomb)
    per_core = []
    for k in range(NCORES):
        af = np.repeat(np.arange(k * CPC, (k + 1) * CPC), P).astype(np.int16)
        d = {"adix": _wrap16(af)}
        for cv, banks in (("1", c1), ("2", c2)):
            for bn, (idxw, dl2, _kbu) in zip("ab", banks):
                d["ix" + cv + bn] = idxw[k]
                d["dl" + cv + bn] = dl2[k]
        per_core.append(d)
    kbus = tuple(banks[b][2] for banks in (c1, c2) for b in (0, 1))
    return common, per_core, kbus


# ------------------------------------------------------------- device program
@with_exitstack
def _emit(ctx, tc, outs, ins, kbus):
    nc = tc.nc
    out_d = outs["out"]
    kbu1a, kbu1b, kbu2a, kbu2b = kbus

    u_d = nc.dram_tensor("U_tab", [NT, 2 * P], f16, kind="Internal").ap()
    ad_ds = {cv: nc.dram_tensor(f"ad{cv}", [NT, 1], f16,
                                kind="Internal").ap() for cv in "12"}

    bos = {}
    for nm, kbu in (("1a", kbu1a), ("1b", kbu1b), ("2a", kbu2a),
                    ("2b", kbu2b)):
        bo = np.zeros(CPC + 1, np.int64)
        bo[1:] = np.cumsum(kbu)
        bos[nm] = bo
    # ---------------- phase A: tables (U replicated; A2 core-local) --------
    with tc.tile_pool(name="pa", bufs=2) as pa, \
         tc.tile_pool(name="pa1", bufs=1) as pa1, \
         tc.tile_pool(name="pap", bufs=3, space="PSUM") as pap:
        wf = pa1.tile([P, 192], f16)
        nc.sync.dma_start(out=wf[:], in_=ins["wfull"][:])
        adw = pa1.tile([P, 2], f16)
        nc.sync.dma_start(out=adw[:], in_=ins["adw"][:])
        # manually double-buffered staging tiles (junk/ones init once);
        # WST rounds are accumulated per table write to amortize HWDGE.
        WST = 4
        ustage = []
        astage = []
        for i in range(2):
            t = pa1.tile([P, WST * RND * 2 * P], f16, tag=f"ust{i}")
            nc.vector.memset(t[:], 0.0)
            tv = t[:].rearrange("p (q c) -> p q c", q=WST * RND)
            nc.vector.memset(tv[:, :, 64], 1.0)
            nc.vector.memset(tv[:, :, 192], 1.0)
            ustage.append(t)
            ta = pa1.tile([2, WST * RND * P], f16, tag=f"ast{i}")
            nc.vector.memset(ta[:], 0.0)
            astage.append(ta)
        u_view = u_d.rearrange("(g p) c -> p g c", p=P)
        ad_views = {cv: ad_ds[cv].rearrange("(g p) c -> g (p c)", p=P)
                    for cv in "12"}
        for piece in range(NCORES):
            xt = pa.tile([P, NPC], f16, tag="xt")
            nc.sync.dma_start(
                out=xt[:], in_=ins["xT"][:, piece * NPC:(piece + 1) * NPC])
            grp = 0
            for j0 in range(0, CPC, WST * RND):
                gr = min(WST * RND, CPC - j0)           # chunks this group
                ut = ustage[grp % 2]
                at = astage[grp % 2]
                grp += 1
                uv = ut[:].rearrange("p (q c) -> p q c", q=WST * RND)
                for q0 in range(0, gr, RND):
                    r = min(RND, gr - q0)
                    ps = pap.tile([P, 192 * RND], f32, tag="pap")
                    for q in range(r):
                        nc.tensor.matmul(
                            out=ps[:, 192 * q:192 * (q + 1)],
                            lhsT=xt[:, (j0 + q0 + q) * P:
                                    (j0 + q0 + q + 1) * P],
                            rhs=wf[:], start=True, stop=True)
                    psv = ps[:].rearrange("p (q c) -> p q c", q=RND)
                    # xw1|xw2 -> U columns 0:64 / 128:192, one strided copy
                    uq = uv[:, q0:q0 + r, :].rearrange(
                        "p q (h c) -> p q h c", h=2)[:, :, :, 0:64]
                    pq = psv[:, :r, :].rearrange(
                        "p q (h c) -> p q h c", h=3)[:, :, 0:2, :]
                    nc.scalar.activation(
                        out=uq, in_=pq,
                        func=mybir.ActivationFunctionType.Copy)
                    for ci in range(2):
                        nc.vector.tensor_copy(
                            out=uv[:, q0:q0 + r, 128 * ci + SASCOL],
                            in_=psv[:, :r, 128 + 2 * ci])
                    # per-chunk a_d rows (nodes on the free axis) for adbc
                    psa = pap.tile([2, RND * P], f32, tag="psa")
                    nc.tensor.matmul(out=psa[:, :r * P], lhsT=adw[:],
                                     rhs=xt[:, (j0 + q0) * P:
                                            (j0 + q0 + r) * P],
                                     start=True, stop=True)
                    nc.vector.tensor_copy(
                        out=at[:, q0 * P:(q0 + r) * P], in_=psa[:, :r * P])
                g0 = piece * CPC + j0
                nc.sync.dma_start(out=u_view[:, g0:g0 + gr, :],
                                  in_=uv[:, :gr, :])
                for ti, cv in enumerate("12"):
                    nc.sync.dma_start(
                        out=ad_views[cv][g0:g0 + gr, :].rearrange(
                            "g c -> (g c)")[None, :],
                        in_=at[ti:ti + 1, :gr * P])

    # ---------------- phase B: edge aggregation ----------------
    with tc.tile_pool(name="pre", bufs=1) as pre, \
         tc.tile_pool(name="pb", bufs=3) as pb, \
         tc.tile_pool(name="pw", bufs=6) as pw, \
         tc.tile_pool(name="pg", bufs=2) as pg, \
         tc.tile_pool(name="pbp", bufs=4, space="PSUM") as pbp:
        iota = pre.tile([P, P], f16)
        nc.sync.dma_start(out=iota[:], in_=ins["iota"][:])
        bcomb = pre.tile([P, DOUT], f32)
        nc.sync.dma_start(out=bcomb[:], in_=ins["bcomb"][:])

        spaces = {}
        for nm, kbu in (("1a", kbu1a), ("1b", kbu1b), ("2a", kbu2a),
                        ("2b", kbu2b)):
            bo = bos[nm]
            maxw = max(int(bo[min(s + SCN, CPC)] - bo[s])
                       for s in range(0, CPC, SCN))
            ci = int(nm[0]) - 1
            base = u_d if (nm[1] == "a" or NT <= BANK) else u_d[BANK:, :]
            tap = base[:, 128 * ci:128 * ci + 128]
            spaces[nm] = dict(kbu=kbu, bo=bo, maxw=maxw, tab=tap)
        dls = {}
        for nm in ("1a", "1b", "2a", "2b"):
            kt = max(sum(spaces[nm]["kbu"]), 1)
            t = pre.tile([P, kt], f32, tag="dl" + nm)
            nc.sync.dma_start(out=t[:], in_=ins["dl" + nm][:])
            dls[nm] = t
        # uniform tile shapes across convs (shared pool tags)
        mwu = {bn: max(spaces["1" + bn]["maxw"], spaces["2" + bn]["maxw"])
               for bn in "ab"}
        kbm = {bn: max(max(spaces["1" + bn]["kbu"]), max(spaces["2" + bn]["kbu"]))
               for bn in "ab"}
        ad_tabs = {cv: ad_ds[cv].rearrange("(g p) c -> g (p c)", p=P)
                   for cv in "12"}
        adix = pre.tile([P, CPC * 8], i16)
        nc.sync.dma_start(out=adix[:], in_=ins["adix"][:])
        # reduce-engine load balancing (blocks assigned so far / target share)
        rshare = {"act": [0, 0.55], "dve": [0, 0.45]}

        def pick_route(kb):
            tot = sum(v[0] for v in rshare.values()) + 1e-9
            r = min(rshare, key=lambda k: rshare[k][0] / (tot * rshare[k][1]))
            rshare[r][0] += kb
            return r

        if VARIANT == "phasea":
            for c in range(CPC):
                nc.sync.dma_start(out=out_d[c * P:(c + 1) * P, :],
                                  in_=bcomb[:])
            return

        for si, sc in enumerate(range(0, CPC, SCN)):
            scr = min(SCN, CPC - sc)
            sc_g = {}
            adbcs = {}
            for cv in "12":
                t = pg.tile([P, SCN, P], f16, tag="adbc" + cv)
                nc.gpsimd.dma_gather(
                    out_ap=t[:, :scr, :], in_ap=ad_tabs[cv],
                    idxs_ap=adix[:, sc * 8:(sc + scr) * 8],
                    num_idxs=scr * P, num_idxs_reg=scr * P, elem_size=P)
                adbcs[cv] = t
                for bn in "ab":
                    nm = cv + bn
                    sp = spaces[nm]
                    b0, b1 = int(sp["bo"][sc]), int(sp["bo"][sc + scr])
                    bw = b1 - b0
                    if bw == 0:
                        sc_g[nm] = (None, b0)
                        continue
                    ixt = pg.tile([P, mwu[bn] * 8], i16, tag="ix" + nm)
                    nc.sync.dma_start(
                        out=ixt[:, :bw * 8],
                        in_=ins["ix" + nm][:, b0 * 8:b1 * 8])
                    gt = pg.tile([P, mwu[bn], P], f16, tag="gt" + bn)
                    for g0 in range(0, bw, GMAXB):
                        gw = min(GMAXB, bw - g0)
                        nc.gpsimd.dma_gather(
                            out_ap=gt[:, g0:g0 + gw, :], in_ap=sp["tab"],
                            idxs_ap=ixt[:, g0 * 8:(g0 + gw) * 8],
                            num_idxs=gw * P, num_idxs_reg=gw * P,
                            elem_size=P, elem_step=2 * P)
                    sc_g[nm] = (gt, b0)

            if VARIANT == "gathers":
                for c in range(sc, sc + scr):
                    nc.sync.dma_start(out=out_d[c * P:(c + 1) * P, :],
                                      in_=bcomb[:])
                continue
            for c in range(sc, sc + scr):
                o1 = None
                psums = {}
                for ci, cv in enumerate("12"):
                    ps = pbp.tile([P, 65], f32, tag="ps" + cv)
                    psums[cv] = ps
                    started = False
                    acts = [(bn, spaces[cv + bn]) for bn in "ab"
                            if int(spaces[cv + bn]["kbu"][c]) > 0]
                    for bi, (bn, sp) in enumerate(acts):
                        nm = cv + bn
                        kb = int(sp["kbu"][c])
                        gt, b0 = sc_g[nm]
                        cj = int(sp["bo"][c]) - b0
                        gts = gt[:, cj:cj + kb, :]
                        dlv = dls[nm]
                        adbc = adbcs[cv][:, c - sc, :]
                        # one-hot blocks (4x-rate fused build, batched tile)
                        ohb = pb.tile([P, kbm[bn], P], f16, tag="ohb" + bn)
                        for j in range(kb):
                            nc.vector.tensor_scalar(
                                out=ohb[:, j, :], in0=iota[:],
                                scalar1=dlv[:, int(sp["bo"][c]) + j:
                                            int(sp["bo"][c]) + j + 1],
                                scalar2=None,
                                op0=mybir.AluOpType.is_equal)
                        # u = a_s + sum(onehot * a_d): batched mult+reduce
                        ohad = pb.tile([P, kbm[bn], P], f16, tag="ohad" + bn)
                        a1 = adbc.unsqueeze(1)
                        nc.vector.tensor_tensor(
                            out=ohad[:, :kb, :], in0=ohb[:, :kb, :],
                            in1=bass.AP(a1.tensor, a1.offset,
                                        [a1.ap[0], [0, kb], a1.ap[2]]),
                            op=mybir.AluOpType.mult)
                        u = pw.tile([P, mwu[bn]], f32, tag="u" + bn)
                        route = pick_route(kb)
                        if route == "act":
                            ascr = pw.tile([P, P], f16, tag="ascr" + bn)
                            for j in range(kb):
                                nc.scalar.activation(
                                    out=ascr[:], in_=ohad[:, j, :],
                                    func=mybir.ActivationFunctionType.Copy,
                                    accum_out=u[:, j:j + 1])
                        else:
                            nc.vector.tensor_reduce(
                                out=u[:, :kb], in_=ohad[:, :kb, :],
                                axis=mybir.AxisListType.X,
                                op=mybir.AluOpType.add)
                        nc.vector.tensor_tensor(
                            out=u[:, :kb], in0=u[:, :kb],
                            in1=gts[:, :, SASCOL], op=mybir.AluOpType.add)
                        # exp(lrelu(u)) == max(exp(u), exp(slope*u))
                        e1 = pw.tile([P, mwu[bn]], f32, tag="e1" + bn)
                        nc.scalar.activation(
                            out=e1[:, :kb], in_=u[:, :kb],
                            func=mybir.ActivationFunctionType.Exp)
                        e2 = pw.tile([P, mwu[bn]], f32, tag="e2" + bn)
                        nc.scalar.activation(
                            out=e2[:, :kb], in_=u[:, :kb],
                            func=mybir.ActivationFunctionType.Exp,
                            scale=NEG_SLOPE)
                        w = pw.tile([P, mwu[bn]], f32, tag="w" + bn)
                        nc.vector.tensor_tensor(
                            out=w[:, :kb], in0=e1[:, :kb], in1=e2[:, :kb],
                            op=mybir.AluOpType.max)
                        # messages [w*xw | w] (ones column scales to w)
                        msg = pb.tile([P, kbm[bn], 65], f16, tag="msg" + bn)
                        nc.vector.tensor_tensor(
                            out=msg[:, :kb, :], in0=gts[:, :, 0:65],
                            in1=w[:, :kb].unsqueeze(-1).to_broadcast(
                                [P, kb, 65]),
                            op=mybir.AluOpType.mult)
                        for j in range(kb):
                            nc.tensor.matmul(
                                out=ps[:], lhsT=ohb[:, j, :],
                                rhs=msg[:, j, :],
                                start=not started,
                                stop=(bi == len(acts) - 1 and j == kb - 1))
                            started = True
                # finalize
                for cv in "12":
                    ps = psums[cv]
                    den = pb.tile([P, 1], f32, tag="den")
                    nc.vector.tensor_scalar_max(den[:], ps[:, 64:65], 1e-30)
                    rec = pb.tile([P, 1], f32, tag="rec")
                    nc.vector.reciprocal(out=rec[:], in_=den[:])
                    rec2 = pb.tile([P, 1], f32, tag="rec2")
                    nc.vector.tensor_scalar_mul(
                        rec2[:], rec[:], (1.0 - ALPHA) if cv == "1" else ALPHA)
                    o = pb.tile([P, DOUT], f32, tag="o" + cv)
                    nc.vector.tensor_scalar(
                        out=o[:], in0=ps[:, 0:64], scalar1=rec2[:],
                        scalar2=None, op0=mybir.AluOpType.mult)
                    if cv == "1":
                        o1 = o
                ofin = pb.tile([P, DOUT], f32, tag="ofin")
                nc.vector.tensor_tensor(
                    out=ofin[:], in0=o1[:], in1=o[:], op=mybir.AluOpType.add)
                nc.vector.tensor_tensor(
                    out=ofin[:], in0=ofin[:], in1=bcomb[:],
                    op=mybir.AluOpType.add)
                nc.sync.dma_start(out=out_d[c * P:(c + 1) * P, :], in_=ofin[:])


def _build(kbus):
    nc = bacc.Bacc("TRN2", target_bir_lowering=False, debug=False,
                   num_devices=NCORES,
                   dynamic_dma_scratch_size=DMA_SCRATCH)
    kbu1a, kbu1b, kbu2a, kbu2b = kbus
    ins = {
        "xT": nc.dram_tensor("xT", [DIN, NT], f16, kind="ExternalInput").ap(),
        "adix": nc.dram_tensor("adix", [P, CPC * 8], i16,
                               kind="ExternalInput").ap(),
        "adw": nc.dram_tensor("adw", [DIN, 2], f16,
                              kind="ExternalInput").ap(),
        "wfull": nc.dram_tensor("wfull", [DIN, 192], f16,
                                kind="ExternalInput").ap(),
        "iota": nc.dram_tensor("iota", [P, P], f16, kind="ExternalInput").ap(),
        "bcomb": nc.dram_tensor("bcomb", [P, DOUT], f32,
                                kind="ExternalInput").ap(),
    }
    for nm, kbu in (("1a", kbu1a), ("1b", kbu1b), ("2a", kbu2a),
                    ("2b", kbu2b)):
        kt = max(sum(kbu), 1)
        ins["ix" + nm] = nc.dram_tensor(
            "ix" + nm, [P, kt * 8], i16, kind="ExternalInput").ap()
        ins["dl" + nm] = nc.dram_tensor(
            "dl" + nm, [P, kt], f32, kind="ExternalInput").ap()
    outs = {"out": nc.dram_tensor("out", [NPC, DOUT], f32,
                                  kind="ExternalOutput").ap()}
    with tile.TileContext(nc) as tc:
        _emit(tc, outs, ins, kbus)
    nc.compile()
    return nc


# ------------------------------------------------------------------- entry
def kernel(x, edge_index, W1, att_src1, att_dst1, b1,
           W2, att_src2, att_dst2, b2):
    common, per_core, kbus = _preprocess(
        np.asarray(x), np.asarray(edge_index),
        np.asarray(W1, np.float64), np.asarray(att_src1, np.float64),
        np.asarray(att_dst1, np.float64), np.asarray(b1, np.float32),
        np.asarray(W2, np.float64), np.asarray(att_src2, np.float64),
        np.asarray(att_dst2, np.float64), np.asarray(b2, np.float32))

    if kbus not in _CACHE:
        _CACHE[kbus] = _build(kbus)
    nc = _CACHE[kbus]

    in_maps = [dict(common, **pc) for pc in per_core]
    res = bass_utils.run_bass_kernel_spmd(
        nc, in_maps, core_ids=list(range(NCORES)))
    full = np.concatenate(
        [res.results[k]["out"] for k in range(NCORES)], axis=0)
    return np.ascontiguousarray(full[:N]).astype(np.float32)
